# revision 9
# baseline (speedup 1.0000x reference)
# Trainium2 Bass kernel: 3-level inverse 2D Haar DWT (DWTInverse, db1, mode=zero).
#
# Math: for a 2-tap synthesis pair (g0=[u0,u1], g1=[v0,v1]) the transposed convs
# have stride 2 and no overlap, so each level is an independent 2x2 butterfly:
#   out[2i+a, 2j+b] = ga[?]... concretely with Haar (u0=u1=v0=a, v1=-a):
#   out[2i,2j]     = 0.5*(ll+lh+hl+hh)
#   out[2i,2j+1]   = 0.5*(ll+lh-hl-hh)
#   out[2i+1,2j]   = 0.5*(ll-lh+hl-hh)
#   out[2i+1,2j+1] = 0.5*(ll-lh-hl+hh)
# Shapes (64->128->256->512 with matching yh sizes) never trigger the crop branch.
#
# Sharding: pure data parallel over batch N=8 -> core k processes n=k
# (32 channels x full spatial). No cross-core communication.
#
# Layout per core: SBUF partition p = (c, b) = c*4+b, c in [0,32) channels,
# b in [0,4) row-blocks of each image. Row-blocks are butterfly-invariant
# (input rows of block b map to output rows of block b), so every level is
# purely free-dim work: elementwise fused scale+add (scalar_tensor_tensor)
# with strided writes doing the 2x2 spatial interleave for free.
#
# Scale folding: carry intermediate ll at scale sigma (llB: u0^4=0.25,
# llC: u0^2=0.5) so that every op is a single fused stt; only yl needs one
# tiny pre-scale by u0^6=0.125.

import numpy as np
from contextlib import ExitStack

C_PER_CORE = 32
N_CORES = 8

_cache = {}


def _build_program(u0, u1, v0, v1):
    import concourse.bacc as bacc
    import concourse.mybir as mybir
    import concourse.tile as tile

    f32 = mybir.dt.float32
    mult = mybir.AluOpType.mult
    add = mybir.AluOpType.add

    rA = v0 / u0  # +1 for Haar (even taps)
    rB = v1 / u1  # -1 for Haar (odd taps)

    # Bacc (not bass.Bass): its compile() runs generate_event_semaphores,
    # which splits multi-sem waits — TPB instructions can carry only ONE
    # sync wait, and walrus codegen hard-errors otherwise.
    nc = bacc.Bacc(
        "TRN2",
        target_bir_lowering=False,
        debug=False,
        enable_asserts=False,
        num_devices=N_CORES,
    )
    yl_t = nc.dram_tensor("yl", [C_PER_CORE, 64, 64], f32, kind="ExternalInput")
    yh0_t = nc.dram_tensor("yh0", [C_PER_CORE, 3, 256, 256], f32, kind="ExternalInput")
    yh1_t = nc.dram_tensor("yh1", [C_PER_CORE, 3, 128, 128], f32, kind="ExternalInput")
    yh2_t = nc.dram_tensor("yh2", [C_PER_CORE, 3, 64, 64], f32, kind="ExternalInput")
    out_t = nc.dram_tensor("out", [C_PER_CORE, 512, 512], f32, kind="ExternalOutput")

    with ExitStack() as ctx:
        tc = ctx.enter_context(tile.TileContext(nc))
        res = ctx.enter_context(tc.tile_pool(name="res", bufs=1))
        yh_pool = ctx.enter_context(tc.tile_pool(name="yh", bufs=2))
        abcd = ctx.enter_context(tc.tile_pool(name="abcd", bufs=2))
        outp = ctx.enter_context(tc.tile_pool(name="outp", bufs=2))

        # Resident ll tiles (per partition: rows of my block, dense row-major).
        llA = res.tile([128, 16 * 64], f32, name="llA")     # 0.125 * yl
        llB = res.tile([128, 32 * 128], f32, name="llB")    # 0.25 * level-A out
        llC = res.tile([128, 64 * 256], f32, name="llC")    # 0.5  * level-B out

        # Load yl and pre-scale by u0^6 (= 0.125 for Haar).
        yl_tmp = yh_pool.tile([128, 16 * 64], f32, name="yl_tmp", tag="yl_tmp", bufs=1)
        yl_v = yl_t[:, :, :].rearrange("c (b r) w -> (c b) r w", b=4)
        nc.sync.dma_start(
            out=yl_tmp.rearrange("p (r w) -> p r w", w=64), in_=yl_v
        )
        nc.scalar.mul(llA[:, :], yl_tmp[:, :], float(u0**6))

        def level(yh_t, H, W, R, sigma, ll_in, dst_tile, dst_dram):
            """One IDWT level. ll_in holds sigma * ll_true, per-partition
            rows_pp = H//4 rows x W cols dense. Writes (sigma/u0^2) * out
            into dst_tile (resident SBUF, 2*rows_pp x 2W) or, for the last
            level, exact outputs staged per chunk then DMA'd to dst_dram."""
            rows_pp = H // 4
            # DRAM view per detail channel k: [c, b, r, w]; the row slice
            # r0:r0+R is contiguous within a block, so each DMA balances to
            # 3 dims: [c(32), b(4), contiguous run].
            yh_v = yh_t[:, :, :, :].rearrange("c k (b r) w -> c k b r w", b=4)
            ll_in_v = ll_in.rearrange("p (r w) -> p r w", w=W)
            if dst_tile is not None:
                dst_v = dst_tile.rearrange(
                    "p (r ar w ac) -> p ar ac r w", ar=2, ac=2, w=W
                )
            else:
                out_dram_v = dst_dram[:, :, :].rearrange("c (b r) w -> (c b) r w", b=4)

            for r0 in range(0, rows_pp, R):
                # For Haar rA=+1, rB=-1: everything is plain add/sub once the
                # yh tile is pre-scaled by sigma. The fused scalar_tensor_tensor
                # op can't be used here: its custom DVE instruction struct only
                # fits ONE sync-wait and walrus refuses multi-wait STTs.
                # Pre-scaling on ACT also collapses the 3 per-k DMA semaphores
                # into a single ACT semaphore for all downstream consumers.
                assert abs(rA - 1.0) < 1e-6 and abs(rB + 1.0) < 1e-6
                yh_tile = yh_pool.tile([128, 3 * R * W], f32, name="yh_tile", tag="yh")
                yh3 = yh_tile.rearrange("p (k r w) -> p k r w", k=3, r=R)
                for k in range(3):
                    nc.sync.dma_start(
                        out=yh3[:, k], in_=yh_v[:, k, :, r0 : r0 + R, :]
                    )
                for k in range(3):  # in-place sigma scale on ACT
                    nc.scalar.mul(yh3[:, k], yh3[:, k], float(sigma))
                lh = yh3[:, 0]
                hl = yh3[:, 1]
                hh = yh3[:, 2]
                ll = ll_in_v[:, r0 : r0 + R, :]

                A = abcd.tile([128, R * W], f32, name="A", tag="A")
                B = abcd.tile([128, R * W], f32, name="B", tag="B")
                Cc = abcd.tile([128, R * W], f32, name="Cc", tag="Cc")
                D = abcd.tile([128, R * W], f32, name="D", tag="D")
                A3 = A.rearrange("p (r w) -> p r w", w=W)
                B3 = B.rearrange("p (r w) -> p r w", w=W)
                C3 = Cc.rearrange("p (r w) -> p r w", w=W)
                D3 = D.rearrange("p (r w) -> p r w", w=W)

                # Height pass: A = sigma*(ll+lh) (even rows), B = sigma*(ll-lh)
                # (odd rows); ll already carries sigma, lh' = sigma*lh.
                sub = mybir.AluOpType.subtract
                nc.vector.tensor_tensor(A3, ll, lh, add)
                nc.vector.tensor_tensor(B3, ll, lh, sub)
                # C = sigma*(hl+hh), D = sigma*(hl-hh) on gpsimd (own SBUF
                # port; DVE 1x-mode fp32 ops don't contend).
                nc.gpsimd.tensor_tensor(C3, hl, hh, add)
                nc.gpsimd.tensor_tensor(D3, hl, hh, sub)

                # Width pass with 2x2 interleaved strided writes.
                if dst_tile is not None:
                    dE = dst_v[:, 0, 0, r0 : r0 + R, :]
                    dF = dst_v[:, 0, 1, r0 : r0 + R, :]
                    dG = dst_v[:, 1, 0, r0 : r0 + R, :]
                    dH = dst_v[:, 1, 1, r0 : r0 + R, :]
                else:
                    ot = outp.tile([128, 2 * R * 2 * W], f32, name="ot", tag="ot")
                    ot_v = ot.rearrange("p (r ar w ac) -> p ar ac r w", ar=2, ac=2, w=W)
                    dE = ot_v[:, 0, 0]
                    dF = ot_v[:, 0, 1]
                    dG = ot_v[:, 1, 0]
                    dH = ot_v[:, 1, 1]

                # Width pass: plain butterflies; output carries 2*sigma relative
                # to true (the next level's ll convention), exactly 1.0 at the
                # final level (sigma=0.5).
                nc.vector.tensor_tensor(dE, A3, C3, add)
                nc.vector.tensor_tensor(dF, A3, C3, sub)
                nc.vector.tensor_tensor(dG, B3, D3, add)
                nc.vector.tensor_tensor(dH, B3, D3, sub)

                if dst_tile is None:
                    nc.scalar.dma_start(
                        out=out_dram_v[:, 2 * r0 : 2 * r0 + 2 * R, :],
                        in_=ot.rearrange("p (r w) -> p r w", w=2 * W),
                    )

        level(yh2_t, 64, 64, 16, float(u0**6), llA, llB, None)
        level(yh1_t, 128, 128, 8, float(u0**4), llB, llC, None)
        level(yh0_t, 256, 256, 4, float(u0**2), llC, None, out_t)

    nc.compile()
    return nc


def _get_nc(u0, u1, v0, v1):
    key = (round(u0, 9), round(u1, 9), round(v0, 9), round(v1, 9))
    if key not in _cache:
        _cache[key] = _build_program(u0, u1, v0, v1)
    return _cache[key]


def _run(inputs, trace=False, trace_kwargs=None):
    from concourse.bass_utils import run_bass_kernel_spmd

    yl = np.ascontiguousarray(np.asarray(inputs["yl"], dtype=np.float32))
    yh0 = np.ascontiguousarray(np.asarray(inputs["yh0"], dtype=np.float32))
    yh1 = np.ascontiguousarray(np.asarray(inputs["yh1"], dtype=np.float32))
    yh2 = np.ascontiguousarray(np.asarray(inputs["yh2"], dtype=np.float32))
    g0 = np.asarray(inputs["g0"], dtype=np.float32)
    g1 = np.asarray(inputs["g1"], dtype=np.float32)

    u0, u1 = float(g0[0]), float(g0[1])
    v0, v1 = float(g1[0]), float(g1[1])

    nc = _get_nc(u0, u1, v0, v1)

    in_maps = [
        {"yl": yl[k], "yh0": yh0[k], "yh1": yh1[k], "yh2": yh2[k]}
        for k in range(N_CORES)
    ]
    kw = {}
    if trace:
        kw["trace"] = True
        if trace_kwargs:
            kw.update(trace_kwargs)
    res = run_bass_kernel_spmd(nc, in_maps, list(range(N_CORES)), **kw)
    out = np.stack([res.results[k]["out"] for k in range(N_CORES)], axis=0)
    return out.astype(np.float32, copy=False), res


def kernel(yl, yh0, yh1, yh2, g0, g1):
    out, _ = _run(
        {"yl": yl, "yh0": yh0, "yh1": yh1, "yh2": yh2, "g0": g0, "g1": g1}
    )
    return out


# revision 12
# speedup vs baseline: 1.6484x; 1.6484x over previous
# Trainium2 Bass kernel: 3-level inverse 2D Haar DWT (DWTInverse, db1, mode=zero).
#
# Math: for a 2-tap synthesis pair (g0=[u0,u1], g1=[v0,v1]) the transposed convs
# have stride 2 and no overlap, so each level is an independent 2x2 butterfly:
#   out[2i+a, 2j+b] = ga[?]... concretely with Haar (u0=u1=v0=a, v1=-a):
#   out[2i,2j]     = 0.5*(ll+lh+hl+hh)
#   out[2i,2j+1]   = 0.5*(ll+lh-hl-hh)
#   out[2i+1,2j]   = 0.5*(ll-lh+hl-hh)
#   out[2i+1,2j+1] = 0.5*(ll-lh-hl+hh)
# Shapes (64->128->256->512 with matching yh sizes) never trigger the crop branch.
#
# Sharding: pure data parallel over batch N=8 -> core k processes n=k
# (32 channels x full spatial). No cross-core communication.
#
# Layout per core: SBUF partition p = (c, b) = c*4+b, c in [0,32) channels,
# b in [0,4) row-blocks of each image. Row-blocks are butterfly-invariant
# (input rows of block b map to output rows of block b), so every level is
# purely free-dim work: elementwise fused scale+add (scalar_tensor_tensor)
# with strided writes doing the 2x2 spatial interleave for free.
#
# Scale folding: carry intermediate ll at scale sigma (llB: u0^4=0.25,
# llC: u0^2=0.5) so that every op is a single fused stt; only yl needs one
# tiny pre-scale by u0^6=0.125.

import numpy as np
from contextlib import ExitStack

C_PER_CORE = 32
N_CORES = 8

_cache = {}


def _build_program(u0, u1, v0, v1, reps=1):
    import concourse.bacc as bacc
    import concourse.mybir as mybir
    import concourse.tile as tile

    f32 = mybir.dt.float32
    mult = mybir.AluOpType.mult
    add = mybir.AluOpType.add

    rA = v0 / u0  # +1 for Haar (even taps)
    rB = v1 / u1  # -1 for Haar (odd taps)

    # Bacc (not bass.Bass): its compile() runs generate_event_semaphores,
    # which splits multi-sem waits — TPB instructions can carry only ONE
    # sync wait, and walrus codegen hard-errors otherwise.
    nc = bacc.Bacc(
        "TRN2",
        target_bir_lowering=False,
        debug=False,
        enable_asserts=False,
        num_devices=N_CORES,
    )
    yl_t = nc.dram_tensor("yl", [C_PER_CORE, 64, 64], f32, kind="ExternalInput")
    yh0_t = nc.dram_tensor("yh0", [C_PER_CORE, 3, 256, 256], f32, kind="ExternalInput")
    yh1_t = nc.dram_tensor("yh1", [C_PER_CORE, 3, 128, 128], f32, kind="ExternalInput")
    yh2_t = nc.dram_tensor("yh2", [C_PER_CORE, 3, 64, 64], f32, kind="ExternalInput")
    out_t = nc.dram_tensor("out", [C_PER_CORE, 512, 512], f32, kind="ExternalOutput")

    with ExitStack() as ctx:
        tc = ctx.enter_context(tile.TileContext(nc))
        res = ctx.enter_context(tc.tile_pool(name="res", bufs=1))
        yh_pool = ctx.enter_context(tc.tile_pool(name="yh", bufs=2))
        abcd = ctx.enter_context(tc.tile_pool(name="abcd", bufs=2))
        outp = ctx.enter_context(tc.tile_pool(name="outp", bufs=2))

        # Resident ll tiles (per partition: rows of my block, dense row-major).
        llA = res.tile([128, 16 * 64], f32, name="llA")     # 0.125 * yl
        llB = res.tile([128, 32 * 128], f32, name="llB")    # 0.25 * level-A out
        llC = res.tile([128, 64 * 256], f32, name="llC")    # 0.5  * level-B out

        def level(yh_t, H, W, R, sigma, ll_in, dst_tile, dst_dram):
            """One IDWT level. ll_in holds sigma * ll_true, per-partition
            rows_pp = H//4 rows x W cols dense. Writes (sigma/u0^2) * out
            into dst_tile (resident SBUF, 2*rows_pp x 2W) or, for the last
            level, exact outputs staged per chunk then DMA'd to dst_dram."""
            rows_pp = H // 4
            # DRAM view per detail channel k: [c, b, r, w]; the row slice
            # r0:r0+R is contiguous within a block, so each DMA balances to
            # 3 dims: [c(32), b(4), contiguous run].
            yh_v = yh_t[:, :, :, :].rearrange("c k (b r) w -> c k b r w", b=4)
            ll_in_v = ll_in.rearrange("p (r w) -> p r w", w=W)
            if dst_tile is not None:
                dst_v = dst_tile.rearrange(
                    "p (r ar w ac) -> p ar ac r w", ar=2, ac=2, w=W
                )
            else:
                out_dram_v = dst_dram[:, :, :].rearrange("c (b r) w -> (c b) r w", b=4)

            for r0 in range(0, rows_pp, R):
                # For Haar rA=+1, rB=-1: everything is plain add/sub once the
                # yh tile is pre-scaled by sigma. The fused scalar_tensor_tensor
                # op can't be used here: its custom DVE instruction struct only
                # fits ONE sync-wait and walrus refuses multi-wait STTs.
                # Pre-scaling on ACT also collapses the 3 per-k DMA semaphores
                # into a single ACT semaphore for all downstream consumers.
                assert abs(rA - 1.0) < 1e-6 and abs(rB + 1.0) < 1e-6
                yh_tile = yh_pool.tile([128, 3 * R * W], f32, name="yh_tile", tag="yh")
                yh3 = yh_tile.rearrange("p (k r w) -> p k r w", k=3, r=R)
                for k in range(3):
                    nc.sync.dma_start(
                        out=yh3[:, k], in_=yh_v[:, k, :, r0 : r0 + R, :]
                    )
                for k in range(3):  # in-place sigma scale on ACT
                    nc.scalar.mul(yh3[:, k], yh3[:, k], float(sigma))
                lh = yh3[:, 0]
                hl = yh3[:, 1]
                hh = yh3[:, 2]
                ll = ll_in_v[:, r0 : r0 + R, :]

                A = abcd.tile([128, R * W], f32, name="A", tag="A")
                B = abcd.tile([128, R * W], f32, name="B", tag="B")
                Cc = abcd.tile([128, R * W], f32, name="Cc", tag="Cc")
                D = abcd.tile([128, R * W], f32, name="D", tag="D")
                A3 = A.rearrange("p (r w) -> p r w", w=W)
                B3 = B.rearrange("p (r w) -> p r w", w=W)
                C3 = Cc.rearrange("p (r w) -> p r w", w=W)
                D3 = D.rearrange("p (r w) -> p r w", w=W)

                # Height pass: A = sigma*(ll+lh) (even rows), B = sigma*(ll-lh)
                # (odd rows); ll already carries sigma, lh' = sigma*lh.
                sub = mybir.AluOpType.subtract
                nc.vector.tensor_tensor(A3, ll, lh, add)
                nc.vector.tensor_tensor(B3, ll, lh, sub)
                # C = sigma*(hl+hh), D = sigma*(hl-hh) on gpsimd (own SBUF
                # port; DVE 1x-mode fp32 ops don't contend).
                nc.gpsimd.tensor_tensor(C3, hl, hh, add)
                nc.gpsimd.tensor_tensor(D3, hl, hh, sub)

                # Width pass with 2x2 interleaved strided writes.
                if dst_tile is not None:
                    dE = dst_v[:, 0, 0, r0 : r0 + R, :]
                    dF = dst_v[:, 0, 1, r0 : r0 + R, :]
                    dG = dst_v[:, 1, 0, r0 : r0 + R, :]
                    dH = dst_v[:, 1, 1, r0 : r0 + R, :]
                else:
                    ot = outp.tile([128, 2 * R * 2 * W], f32, name="ot", tag="ot")
                    ot_v = ot.rearrange("p (r ar w ac) -> p ar ac r w", ar=2, ac=2, w=W)
                    dE = ot_v[:, 0, 0]
                    dF = ot_v[:, 0, 1]
                    dG = ot_v[:, 1, 0]
                    dH = ot_v[:, 1, 1]

                # Width pass: plain butterflies; output carries 2*sigma relative
                # to true (the next level's ll convention), exactly 1.0 at the
                # final level (sigma=0.5).
                nc.vector.tensor_tensor(dE, A3, C3, add)
                nc.vector.tensor_tensor(dF, A3, C3, sub)
                nc.vector.tensor_tensor(dG, B3, D3, add)
                nc.vector.tensor_tensor(dH, B3, D3, sub)

                if dst_tile is None:
                    nc.scalar.dma_start(
                        out=out_dram_v[:, 2 * r0 : 2 * r0 + 2 * R, :],
                        in_=ot.rearrange("p (r w) -> p r w", w=2 * W),
                    )

        for _ in range(reps):  # reps>1 only for benchmarking (device-side loop)
            # Load yl and pre-scale by u0^6 (= 0.125 for Haar).
            yl_tmp = yh_pool.tile(
                [128, 16 * 64], f32, name="yl_tmp", tag="yl_tmp", bufs=1
            )
            yl_v = yl_t[:, :, :].rearrange("c (b r) w -> (c b) r w", b=4)
            nc.sync.dma_start(
                out=yl_tmp.rearrange("p (r w) -> p r w", w=64), in_=yl_v
            )
            nc.scalar.mul(llA[:, :], yl_tmp[:, :], float(u0**6))

            level(yh2_t, 64, 64, 16, float(u0**6), llA, llB, None)
            level(yh1_t, 128, 128, 8, float(u0**4), llB, llC, None)
            level(yh0_t, 256, 256, 4, float(u0**2), llC, None, out_t)

    nc.compile()
    return nc


def _get_nc(u0, u1, v0, v1):
    key = (round(u0, 9), round(u1, 9), round(v0, 9), round(v1, 9))
    if key not in _cache:
        _cache[key] = _build_program(u0, u1, v0, v1)
    return _cache[key]


def _run(inputs, trace=False, trace_kwargs=None):
    from concourse.bass_utils import run_bass_kernel_spmd

    yl = np.ascontiguousarray(np.asarray(inputs["yl"], dtype=np.float32))
    yh0 = np.ascontiguousarray(np.asarray(inputs["yh0"], dtype=np.float32))
    yh1 = np.ascontiguousarray(np.asarray(inputs["yh1"], dtype=np.float32))
    yh2 = np.ascontiguousarray(np.asarray(inputs["yh2"], dtype=np.float32))
    g0 = np.asarray(inputs["g0"], dtype=np.float32)
    g1 = np.asarray(inputs["g1"], dtype=np.float32)

    u0, u1 = float(g0[0]), float(g0[1])
    v0, v1 = float(g1[0]), float(g1[1])

    nc = _get_nc(u0, u1, v0, v1)

    in_maps = [
        {"yl": yl[k], "yh0": yh0[k], "yh1": yh1[k], "yh2": yh2[k]}
        for k in range(N_CORES)
    ]
    kw = {}
    if trace:
        kw["trace"] = True
        if trace_kwargs:
            kw.update(trace_kwargs)
    res = run_bass_kernel_spmd(nc, in_maps, list(range(N_CORES)), **kw)
    out = np.stack([res.results[k]["out"] for k in range(N_CORES)], axis=0)
    return out.astype(np.float32, copy=False), res


def kernel(yl, yh0, yh1, yh2, g0, g1):
    out, _ = _run(
        {"yl": yl, "yh0": yh0, "yh1": yh1, "yh2": yh2, "g0": g0, "g1": g1}
    )
    return out
